# revision 57
# baseline (speedup 1.0000x reference)
"""Mamba-1 selective scan on 8 Trainium2 NeuronCores — v2.

Sharding: core c -> (batch b = c//2, D-half h = c%2): each core owns 512
channels of one batch for the recurrence; projections need the full D=1024.

Math (exact ZOH, rescaled state):
  G = A + 1e-8,  shat := G * s
  a_t = exp(dt_t * A)                           (per d,n,t)
  shat_t = a_t shat_{t-1} + (a_t - 1) ghat_t,   ghat = x * B
  w := shat + ghat  ->  w_t = (delta_t + w_{t-1}) * a_t,
       delta_t = ghat_t - ghat_{t-1}            (hw tensor_tensor_scan)
  y_t[d] = sum_n (1/G)[d,n] (w - ghat) C[n,t] + Dskip[d] x[d,t]
         = [sum_n diag(1/G_n) @ (w_n * crep_n)]  - x*q + Dskip*x
    q[d,t] = sum_n (1/G)[d,n] B[n,t] C[n,t]     (PE matmul of bc = B*C)

v2 engine plan (vs the v1 PE-broadcast/ACT-copy design):
  - x arrives transposed via XBAR dma_start_transpose (no PE transposes,
    no psum staging copies).
  - B/C rows bounce through a DRAM ring and come back as DMA partition
    broadcasts (no PE broadcast matmuls, no ACT psum->sbuf copies).
  - n is processed in pairs; gt/dl/sct are single [128, 2, TC] tensor ops
    (0-stride broadcast of x over the pair dim).
  - dl/sct alternate pairs between DVE and Pool to balance the two engines;
    scans are DVE-only (ISA).
  - scan carries live in a fp16 wc array updated by tiny DMAs, not ACT.
"""

import sys

import numpy as np

sys.path.insert(0, "/opt/trn_rl_repo")

import concourse.bacc as bacc
import concourse.mybir as mybir
import concourse.tile as tile
from concourse.bass_utils import run_bass_kernel_spmd

B, T, D, N, R = 4, 4096, 1024, 16, 64
NCORES = 8
DH = D // 2            # channels per core
NDT = DH // 128        # d-tiles per core (4)
KD = D // 128          # k-tiles over full D for projections (8)
TC = 1024              # time chunk
NCH = T // TC
PH = 512               # psum piece (one bank of f32)
NPC = TC // PH         # psum pieces per chunk (2)
NPR = N // 2           # n-pairs (8)
F32 = mybir.dt.float32
FP16 = mybir.dt.float16
AL = mybir.AluOpType
AF = mybir.ActivationFunctionType

# pair index sets: which pairs run dl / sct on Pool (else DVE).
# dl stays on DVE (it feeds the scans: keep the DVE chain self-contained);
# sct is a leaf (only the PE reads it) so it all goes to Pool.
DL_POOL = ()
SCT_POOL = (0, 1, 2, 3, 4, 5, 6, 7)

_CACHE = {}


def _patch_act_tables():
    """Route Exp+Ln to natural_log_exp_and_others so the softplus (Exp,Ln)
    and the main-loop Exp never force activation-table reloads."""
    import concourse.bacc as _bacc
    from concourse.hw_specs import get_activation_tables as _orig

    def patched(arch):
        t = _orig(arch)
        exp = mybir.ActivationFunctionType.Exp
        ln = mybir.ActivationFunctionType.Ln
        for name, fns in t.items():
            if name != "natural_log_exp_and_others":
                fns.discard(exp)
                fns.discard(ln)
        return t

    _bacc.get_activation_tables = patched


def _build_program():
    _patch_act_tables()
    nc = bacc.Bacc(
        "TRN2",
        target_bir_lowering=False,
        debug=False,
        num_devices=NCORES,
    )

    x_d = nc.dram_tensor("x16", [T, D], FP16, kind="ExternalInput")
    wall_d = nc.dram_tensor("wall", [128, KD * 112], FP16, kind="ExternalInput")
    w2_d = nc.dram_tensor("w2r", [64, NDT * 128], FP16, kind="ExternalInput")
    bd_d = nc.dram_tensor("bdt2", [128, NDT], F32, kind="ExternalInput")
    ac_d = nc.dram_tensor("acols", [128, NDT * N], F32, kind="ExternalInput")
    dgw_d = nc.dram_tensor("dgw", [128, NDT * N * 128], FP16,
                           kind="ExternalInput")
    dsk_d = nc.dram_tensor("dskw", [128, NDT * 128], FP16,
                           kind="ExternalInput")
    qw_d = nc.dram_tensor("qw", [16, NDT * 128], FP16, kind="ExternalInput")
    nid_d = nc.dram_tensor("nident", [128, 128], FP16, kind="ExternalInput")
    w0_d = nc.dram_tensor("w0init", [128, NDT * N], F32, kind="ExternalInput")
    y_d = nc.dram_tensor("yT", [DH, T], FP16, kind="ExternalOutput")
    # B/C row staging ring in DRAM: per chunk 32 rows
    # rows 0..15: B rows over times t0-1 .. t0+TC-1  ([16, TC+1])
    # rows 16..31: C rows over times t0 .. t0+TC-1   ([16, TC], col TC unused)
    bcst_d = nc.dram_tensor("bcstage", [2 * 32, TC + 1], FP16, kind="Internal")

    with tile.TileContext(nc) as tc:
        _body(tc, x_d, wall_d, w2_d, bd_d, ac_d, dgw_d, dsk_d, qw_d,
              nid_d, w0_d, y_d, bcst_d)

    nc.compile()
    return nc


def _body(tc, x_d, wall_d, w2_d, bd_d, ac_d, dgw_d, dsk_d, qw_d,
          nid_d, w0_d, y_d, bcst_d):
    nc = tc.nc

    with (
        tc.tile_pool(name="const", bufs=1) as const,
        tc.tile_pool(name="xtcp", bufs=2) as xtcp,
        tc.tile_pool(name="xprp", bufs=1) as xprp,
        tc.tile_pool(name="pallcp", bufs=2) as pallcp,
        tc.tile_pool(name="xbp", bufs=2) as xbp,
        tc.tile_pool(name="dtp", bufs=2) as dtp,
        tc.tile_pool(name="bcp", bufs=1) as bcp,
        tc.tile_pool(name="atp", bufs=3) as atp,
        tc.tile_pool(name="gwp", bufs=2) as gwp,
        tc.tile_pool(name="dlpp", bufs=2) as dlpp,
        tc.tile_pool(name="wtpp", bufs=5) as wtpp,
        tc.tile_pool(name="sctpp", bufs=2) as sctpp,
        tc.tile_pool(name="workp", bufs=1) as workp,
        tc.tile_pool(name="qycp", bufs=2) as qycp,
        tc.tile_pool(name="youtp", bufs=1) as youtp,
        tc.tile_pool(name="psY", bufs=2, space="PSUM") as psY,
        tc.tile_pool(name="psQ", bufs=2, space="PSUM") as psQ,
        tc.tile_pool(name="psP", bufs=2, space="PSUM") as psP,
    ):
        # ---- constants ----
        nident = const.tile([128, 128], FP16)
        nc.scalar.dma_start(nident, nid_d[:, :])
        wall = const.tile([128, KD, 112], FP16)
        nc.sync.dma_start(wall, wall_d.ap().rearrange("p (k m) -> p k m",
                                                      k=KD))
        w2r = const.tile([64, NDT, 128], FP16)
        nc.scalar.dma_start(w2r, w2_d.ap().rearrange("p (d m) -> p d m",
                                                     d=NDT))
        bdt2 = const.tile([128, NDT], F32)
        nc.scalar.dma_start(bdt2, bd_d[:, :])
        acols = const.tile([128, NDT * N], F32)
        nc.scalar.dma_start(acols, ac_d[:, :])
        dskw = const.tile([128, NDT, 128], FP16)
        nc.sync.dma_start(dskw, dsk_d.ap().rearrange("p (d m) -> p d m",
                                                     d=NDT))
        qw = const.tile([16, NDT, 128], FP16)
        nc.sync.dma_start(qw, qw_d.ap().rearrange("p (d m) -> p d m",
                                                  d=NDT))
        dgw = const.tile([128, NDT * N, 128], FP16)
        nc.gpsimd.dma_start(dgw, dgw_d.ap().rearrange("p (g m) -> p g m",
                                                      g=NDT * N))
        wc = const.tile([128, NDT * N], F32)
        nc.sync.dma_start(wc, w0_d[:, :])


        stage_prev = {}
        dts_tiles = {}
        deferred = []

        def flush_deferred():
            while deferred:
                deferred.pop(0)()

        def flush_deferred_yo():
            while deferred_yo:
                deferred_yo.pop(0)()

        qsb_pending = {}
        bc_tiles = {}

        def make_bc(ch):
            # realigned B/C rows for the q-trick (partition move 64->0);
            # emitted mid-chunk so the bc TT never head-blocks DVE's stream
            # on the next chunk's staging chain.
            pallc = stage_prev[ch][1]
            btc = workp.tile([16, TC], FP16, tag="btc", name="btc")
            nc.sync.dma_start(btc, pallc[64:80, 1:1 + TC])
            ctc = workp.tile([16, TC], FP16, tag="ctc", name="ctc")
            nc.sync.dma_start(ctc, pallc[96:112, 1:1 + TC])
            bc = workp.tile([16, TC], FP16, tag="bc", name="bc")
            nc.vector.tensor_tensor(bc, btc, ctc, AL.mult)
            bc_tiles[ch] = bc

        def prep_q(ch, dtl):
            # qsb for (ch, dtl): emitted one dtile-pass ahead so the PE
            # matmuls sit mid-stream, never behind a dtile tail.
            bcq = bc_tiles[ch]
            qsb = qycp.tile([128, TC], FP16, tag="qsb", name="qsb")
            for hf in range(NPC):
                pq = psQ.tile([128, PH], F32, tag="psQ")
                nc.tensor.matmul(pq, qw[:, dtl, :],
                                 bcq[:, hf * PH:(hf + 1) * PH],
                                 start=True, stop=True)
                nc.scalar.copy(qsb[:, hf * PH:(hf + 1) * PH], pq)
            qsb_pending[(ch, dtl)] = qsb

        def dt_stage(ch, dtl):
            # dt for one dtile: softplus(w2 @ xr + b)
            if ch not in dts_tiles:
                dts_tiles[ch] = dtp.tile([128, NDT, TC], FP16, tag="dts",
                                         name="dts")
            dts = dts_tiles[ch]
            pallc = stage_prev[ch][1]
            for hf in range(NPC):
                sl = slice(1 + hf * PH, 1 + (hf + 1) * PH)
                pdt = psQ.tile([128, PH], F32, tag="psQ")
                nc.tensor.matmul(pdt, w2r[:, dtl, :], pallc[0:64, sl],
                                 start=True, stop=True)
                dsl = dts[:, dtl, hf * PH:(hf + 1) * PH]
                nc.scalar.activation(dsl, pdt, AF.Exp,
                                     bias=bdt2[:, dtl:dtl + 1], scale=1.0)
            nc.scalar.activation(dts[:, dtl, :], dts[:, dtl, :],
                                 AF.Ln, bias=1.0, scale=1.0)

        def stage(ch):
            """Load + transpose x for chunk ch, run projections, ship B/C
            rows to the DRAM staging ring."""
            t0 = ch * TC
            ring = ch % 2
            xtc = xtcp.tile([128, NDT, TC], FP16, tag="xtc", name="xtc")
            xpr = xprp.tile([128, KD - NDT, TC], FP16, tag="xpr", name="xpr")
            pallc = pallcp.tile([112, TC + 1], FP16, tag="pallc",
                                name="pallc")
            for k in range(KD):
                src = x_d[t0:t0 + TC, k * 128:(k + 1) * 128]
                if k < NDT:
                    nc.sync.dma_start_transpose(xtc[:, k, :], src)
                else:
                    nc.sync.dma_start_transpose(xpr[:, k - NDT, :], src)
            xb = xbp.tile([128, NDT, 1], FP16, tag="xb", name="xb")
            if ch == 0:
                nc.vector.memset(pallc[:, 0:1], 0.0)
                nc.vector.memset(xb, 0.0)
            else:
                xp0, pp0 = stage_prev[ch - 1][0], stage_prev[ch - 1][1]
                nc.scalar.copy(pallc[:, 0:1], pp0[:, TC:TC + 1])
                nc.scalar.copy(xb, xp0[:, :, TC - 1:TC])
            stage_prev[ch] = (xtc, pallc, None, xb)

            for tp in range(NPC):
                pp = psP.tile([112, PH], F32, tag="psP")
                for k in range(KD):
                    if k < NDT:
                        srck = xtc[:, k, tp * PH:(tp + 1) * PH]
                    else:
                        srck = xpr[:, k - NDT, tp * PH:(tp + 1) * PH]
                    nc.tensor.matmul(pp, wall[:, k, :], srck,
                                     start=(k == 0), stop=(k == KD - 1))
                nc.scalar.copy(pallc[:, 1 + tp * PH:1 + (tp + 1) * PH], pp)

            # ship B rows (with t0-1 col) and C rows to the DRAM ring
            nc.scalar.dma_start(bcst_d[ring * 32:ring * 32 + 16, :],
                                pallc[64:80, :])
            nc.scalar.dma_start(bcst_d[ring * 32 + 16:ring * 32 + 32, 0:TC],
                                pallc[96:112, 1:TC + 1])
            stage_prev[ch] = (xtc, pallc, None, xb)

        def bcast_pair(ch, pr):
            """DMA-broadcast B/C rows for pair pr of chunk ch from the DRAM
            ring to all 128 partitions."""
            ring = ch % 2
            brp = bcp.tile([128, 2, TC + 1], FP16, tag=f"brp{pr}",
                           name=f"brp{pr}")
            nc.sync.dma_start(
                brp, bcst_d[ring * 32 + 2 * pr:ring * 32 + 2 * pr + 2, :]
                .unsqueeze(0).broadcast_to([128, 2, TC + 1]))
            crp = bcp.tile([128, 2, TC], FP16, tag=f"crp{pr}",
                           name=f"crp{pr}")
            nc.sync.dma_start(
                crp, bcst_d[ring * 32 + 16 + 2 * pr:ring * 32 + 18 + 2 * pr,
                            0:TC]
                .unsqueeze(0).broadcast_to([128, 2, TC]))
            return brp, crp

        def run_chunk(ch):
            t0 = ch * TC
            xtc, pallc, _, xb = stage_prev[ch]
            dts = dts_tiles[ch]

            pair_tiles = {}
            for dtl in range(NDT):
                py = psY.tile([128, TC], F32, tag="psY", name="py")
                pys = [py[:, hf * PH:(hf + 1) * PH] for hf in range(NPC)]
                # q path: pq matmuls now; qsb copies deferred into ACT slack
                # after the next pair's at-exps; ycr emitted at pr==2 so it
                # does not head-block Pool's sct stream.
                if (ch, dtl) not in qsb_pending:
                    prep_q(ch, dtl)
                qsb = qsb_pending.pop((ch, dtl))
                ycr = qycp.tile([128, TC], FP16, tag="ycr", name="ycr")

                for pr in range(NPR):
                    if dtl == 0:
                        pair_tiles[pr] = bcast_pair(ch, pr)
                    brp, crp = pair_tiles[pr]
                    g0 = dtl * N + 2 * pr

                    ats = []
                    for j in range(2):
                        at = atp.tile([128, TC], F32, tag="at", name="at")
                        nc.scalar.activation(
                            at, dts[:, dtl, :], AF.Exp,
                            scale=acols[:, g0 + j:g0 + j + 1])
                        ats.append(at)
                    if pr == 1:
                        flush_deferred()
                        if dtl == 2 and ch + 1 < NCH:
                            make_bc(ch + 1)
                    elif pr == 3 and dtl == 0 and ch + 1 < NCH:
                        stage(ch + 1)
                    elif pr == 3:
                        flush_deferred_yo()
                    elif pr == 5 and ch + 1 < NCH:
                        dt_stage(ch + 1, dtl)
                    elif pr == 6:
                        if dtl + 1 < NDT:
                            prep_q(ch, dtl + 1)
                        elif ch + 1 < NCH:
                            prep_q(ch + 1, 0)

                    gtp = gwp.tile([128, 2, TC], FP16, tag="gtp",
                                   name="gtp")
                    nc.vector.tensor_tensor(
                        gtp,
                        xtc[:, dtl, :].unsqueeze(1)
                        .broadcast_to([128, 2, TC]),
                        brp[:, :, 1:TC + 1], AL.mult)
                    # boundary gt at time t0-1 from the previous chunk's x
                    gb = gwp.tile([128, 2, 1], FP16, tag="gb", name="gb")
                    nc.vector.tensor_tensor(
                        gb, xb[:, dtl, :].unsqueeze(1)
                        .broadcast_to([128, 2, 1]),
                        brp[:, :, 0:1], AL.mult)
                    if dtl == NDT - 1 and ch + 1 < NCH:
                        pair_tiles[(ch + 1, pr)] = bcast_pair(ch + 1, pr)
                    dlp = dlpp.tile([128, 2, TC], FP16, tag="dlp", name="dlp")
                    dl_eng = nc.gpsimd if pr in DL_POOL else nc.vector
                    nc.vector.tensor_tensor(dlp[:, :, 0:1], gtp[:, :, 0:1],
                                            gb, AL.subtract)
                    dl_eng.tensor_tensor(dlp[:, :, 1:TC], gtp[:, :, 1:TC],
                                         gtp[:, :, 0:TC - 1], AL.subtract)

                    wtp = wtpp.tile([128, 2, TC], FP16, tag="wtp",
                                    name="wtp")
                    for j in range(2):
                        nc.vector.tensor_tensor_scan(
                            wtp[:, j, :], dlp[:, j, :], ats[j],
                            wc[:, g0 + j:g0 + j + 1],
                            op0=AL.add, op1=AL.mult)
                        nc.scalar.copy(wc[:, g0 + j:g0 + j + 1],
                                       wtp[:, j, TC - 1:TC])

                    if pr == 3:
                        nc.gpsimd.tensor_tensor(
                            ycr, xtc[:, dtl, :], qsb, AL.mult)
                    sctp = sctpp.tile([128, 2, TC], FP16, tag="sctp",
                                      name="sctp")
                    last_pass = ch == NCH - 1 and dtl == NDT - 1 and pr >= 6
                    sct_eng = (nc.vector if last_pass else
                               nc.gpsimd if pr in SCT_POOL else nc.vector)
                    sct_eng.tensor_tensor(sctp, wtp, crp, AL.mult)

                    for j in range(2):
                        for hf in range(NPC):
                            nc.tensor.matmul(
                                pys[hf], dgw[:, g0 + j, :],
                                sctp[:, j, hf * PH:(hf + 1) * PH],
                                start=(pr == 0 and j == 0), stop=False)

                # D_skip * x and the -x*q correction
                for hf in range(NPC):
                    nc.tensor.matmul(
                        pys[hf], dskw[:, dtl, :],
                        xtc[:, dtl, hf * PH:(hf + 1) * PH],
                        start=False, stop=False)
                for hf in range(NPC):
                    nc.tensor.matmul(pys[hf], nident,
                                     ycr[:, hf * PH:(hf + 1) * PH],
                                     start=False, stop=True)

                def make_yo(py=py, dtl=dtl, t0=t0):
                    def emit():
                        yo = youtp.tile([128, TC], FP16, tag="yo", name="yo")
                        nc.scalar.copy(yo, py)
                        nc.sync.dma_start(
                            y_d[dtl * 128:(dtl + 1) * 128, t0:t0 + TC], yo)
                    return emit
                deferred_yo.append(make_yo())
                # chunk 0 still hoists its own dt softplus dtile-to-dtile;
                # later chunks were fully precomputed at pr==5 of chunk ch-1.
                if ch == 0 and dtl + 1 < NDT:
                    dt_stage(ch, dtl + 1)

        pe_warmup()
        stage(0)
        make_bc(0)
        dt_stage(0, 0)
        for ch in range(NCH):
            run_chunk(ch)
        flush_deferred()
        flush_deferred_yo()


def kernel(x, state, log_A, W_B, W_C, W_dt1, W_dt2, b_dt2, D_skip):
    if "nc" not in _CACHE:
        _CACHE["nc"] = _build_program()
    nc = _CACHE["nc"]

    x = np.asarray(x, np.float32)
    state = np.asarray(state, np.float32)
    A = (-np.exp(np.asarray(log_A, np.float32))).astype(np.float32)
    G = (A + np.float32(1e-8)).astype(np.float32)
    invG = (np.float32(1.0) / G).astype(np.float32)
    W_B = np.asarray(W_B, np.float32)
    W_C = np.asarray(W_C, np.float32)
    W_dt1 = np.asarray(W_dt1, np.float32)
    W_dt2 = np.asarray(W_dt2, np.float32)
    b_dt2 = np.asarray(b_dt2, np.float32)
    D_skip = np.asarray(D_skip, np.float32)

    nident = (-np.eye(128)).astype(np.float16)

    in_maps = []
    for c in range(NCORES):
        b, h = c // 2, c % 2
        loc = slice(h * DH, (h + 1) * DH)
        perm = np.r_[np.arange(h * DH, (h + 1) * DH),
                     np.arange((1 - h) * DH, (2 - h) * DH)]
        Al = A[loc]                      # [DH, N]
        Gl = G[loc]
        invGl = invG[loc]

        # wall: [W_dt1.T | W_B.T | zeros | W_C.T] with permuted rows
        wallf = np.concatenate(
            [W_dt1.T[perm], W_B.T[perm],
             np.zeros((D, 16), np.float32), W_C.T[perm]], axis=1)  # [D, 112]
        wall = np.ascontiguousarray(
            wallf.reshape(KD, 128, 112).transpose(1, 0, 2).reshape(
                128, KD * 112)).astype(np.float16)

        # w2r: [64, NDT*128]
        w2r = np.ascontiguousarray(
            W_dt2[loc].T.reshape(64, NDT, 128).reshape(64, NDT * 128)
        ).astype(np.float16)

        bd = np.ascontiguousarray(b_dt2[loc].reshape(NDT, 128).T)

        # acols: [128, NDT*N] col (dtl*N+n) = A[dtl*128+p, n]
        acols = np.ascontiguousarray(
            Al.reshape(NDT, 128, N).transpose(1, 0, 2).reshape(128, NDT * N))

        # dgw: diag(invG) per (dtl, n): [128, NDT*N*128]
        dgwm = np.zeros((128, NDT * N, 128), np.float32)
        p = np.arange(128)
        for dtl in range(NDT):
            for n in range(N):
                dgwm[p, dtl * N + n, p] = invGl[dtl * 128 + p, n]
        dgw = np.ascontiguousarray(
            dgwm.reshape(128, NDT * N * 128)).astype(np.float16)

        # dskw: diag(D_skip) per dtl
        dskm = np.zeros((128, NDT, 128), np.float32)
        for dtl in range(NDT):
            dskm[p, dtl, p] = D_skip[loc][dtl * 128 + p]
        dskw = np.ascontiguousarray(
            dskm.reshape(128, NDT * 128)).astype(np.float16)

        # qw: [16, NDT*128]  qw[n, dtl*128+p] = invG[dtl*128+p, n]
        qwm = np.ascontiguousarray(
            invGl.T.reshape(N, NDT, 128).reshape(16, NDT * 128)
        ).astype(np.float16)

        # w0init: G*state0 laid out [128, NDT*N] (fp16 carries)
        w0 = (Gl * state[b, loc]).reshape(NDT, 128, N).transpose(1, 0, 2)
        w0 = np.ascontiguousarray(w0.reshape(128, NDT * N)).astype(np.float32)

        in_maps.append({
            "x16": np.ascontiguousarray(x[b][:, perm]).astype(np.float16),
            "wall": wall,
            "w2r": w2r,
            "bdt2": bd,
            "acols": acols,
            "dgw": dgw,
            "dskw": dskw,
            "qw": qwm,
            "nident": nident,
            "w0init": w0,
        })

    res = run_bass_kernel_spmd(nc, in_maps, core_ids=list(range(NCORES)))

    y = np.empty((B, T, D), np.float32)
    for c in range(NCORES):
        b, h = c // 2, c % 2
        y[b][:, h * DH:(h + 1) * DH] = res.results[c]["yT"].T.astype(
            np.float32)
    return y


# revision 64
# speedup vs baseline: 1.0038x; 1.0038x over previous
"""Mamba-1 selective scan on 8 Trainium2 NeuronCores — v2.

Sharding: core c -> (batch b = c//2, D-half h = c%2): each core owns 512
channels of one batch for the recurrence; projections need the full D=1024.

Math (exact ZOH, rescaled state):
  G = A + 1e-8,  shat := G * s
  a_t = exp(dt_t * A)                           (per d,n,t)
  shat_t = a_t shat_{t-1} + (a_t - 1) ghat_t,   ghat = x * B
  w := shat + ghat  ->  w_t = (delta_t + w_{t-1}) * a_t,
       delta_t = ghat_t - ghat_{t-1}            (hw tensor_tensor_scan)
  y_t[d] = sum_n (1/G)[d,n] (w - ghat) C[n,t] + Dskip[d] x[d,t]
         = [sum_n diag(1/G_n) @ (w_n * crep_n)]  - x*q + Dskip*x
    q[d,t] = sum_n (1/G)[d,n] B[n,t] C[n,t]     (PE matmul of bc = B*C)

v2 engine plan (vs the v1 PE-broadcast/ACT-copy design):
  - x arrives transposed via XBAR dma_start_transpose (no PE transposes,
    no psum staging copies).
  - B/C rows bounce through a DRAM ring and come back as DMA partition
    broadcasts (no PE broadcast matmuls, no ACT psum->sbuf copies).
  - n is processed in pairs; gt/dl/sct are single [128, 2, TC] tensor ops
    (0-stride broadcast of x over the pair dim).
  - dl/sct alternate pairs between DVE and Pool to balance the two engines;
    scans are DVE-only (ISA).
  - scan carries live in a fp16 wc array updated by tiny DMAs, not ACT.
"""

import sys

import numpy as np

sys.path.insert(0, "/opt/trn_rl_repo")

import concourse.bacc as bacc
import concourse.mybir as mybir
import concourse.tile as tile
from concourse.bass_utils import run_bass_kernel_spmd

B, T, D, N, R = 4, 4096, 1024, 16, 64
NCORES = 8
DH = D // 2            # channels per core
NDT = DH // 128        # d-tiles per core (4)
KD = D // 128          # k-tiles over full D for projections (8)
TC = 1024              # time chunk
NCH = T // TC
PH = 512               # psum piece (one bank of f32)
NPC = TC // PH         # psum pieces per chunk (2)
NPR = N // 2           # n-pairs (8)
F32 = mybir.dt.float32
FP16 = mybir.dt.float16
AL = mybir.AluOpType
AF = mybir.ActivationFunctionType

# pair index sets: which pairs run dl / sct on Pool (else DVE).
# dl stays on DVE (it feeds the scans: keep the DVE chain self-contained);
# sct is a leaf (only the PE reads it) so it all goes to Pool.
DL_POOL = ()
SCT_POOL = (0, 1, 2, 3, 4, 5, 6, 7)

_CACHE = {}


def _patch_act_tables():
    """Route Exp+Ln to natural_log_exp_and_others so the softplus (Exp,Ln)
    and the main-loop Exp never force activation-table reloads."""
    import concourse.bacc as _bacc
    from concourse.hw_specs import get_activation_tables as _orig

    def patched(arch):
        t = _orig(arch)
        exp = mybir.ActivationFunctionType.Exp
        ln = mybir.ActivationFunctionType.Ln
        for name, fns in t.items():
            if name != "natural_log_exp_and_others":
                fns.discard(exp)
                fns.discard(ln)
        return t

    _bacc.get_activation_tables = patched


def _build_program():
    _patch_act_tables()
    nc = bacc.Bacc(
        "TRN2",
        target_bir_lowering=False,
        debug=False,
        num_devices=NCORES,
    )

    x_d = nc.dram_tensor("x16", [T, D], FP16, kind="ExternalInput")
    wall_d = nc.dram_tensor("wall", [128, KD * 112], FP16, kind="ExternalInput")
    w2_d = nc.dram_tensor("w2r", [64, NDT * 128], FP16, kind="ExternalInput")
    bd_d = nc.dram_tensor("bdt2", [128, NDT], F32, kind="ExternalInput")
    ac_d = nc.dram_tensor("acols", [128, NDT * N], F32, kind="ExternalInput")
    dgw_d = nc.dram_tensor("dgw", [128, NDT * N * 128], FP16,
                           kind="ExternalInput")
    dsk_d = nc.dram_tensor("dskw", [128, NDT * 128], FP16,
                           kind="ExternalInput")
    qw_d = nc.dram_tensor("qw", [16, NDT * 128], FP16, kind="ExternalInput")
    nid_d = nc.dram_tensor("nident", [128, 128], FP16, kind="ExternalInput")
    w0_d = nc.dram_tensor("w0init", [128, NDT * N], F32, kind="ExternalInput")
    y_d = nc.dram_tensor("yT", [DH, T], FP16, kind="ExternalOutput")
    # B/C row staging ring in DRAM: per chunk 32 rows
    # rows 0..15: B rows over times t0-1 .. t0+TC-1  ([16, TC+1])
    # rows 16..31: C rows over times t0 .. t0+TC-1   ([16, TC], col TC unused)
    bcst_d = nc.dram_tensor("bcstage", [2 * 32, TC + 1], FP16, kind="Internal")

    with tile.TileContext(nc) as tc:
        _body(tc, x_d, wall_d, w2_d, bd_d, ac_d, dgw_d, dsk_d, qw_d,
              nid_d, w0_d, y_d, bcst_d)

    nc.compile()
    return nc


def _body(tc, x_d, wall_d, w2_d, bd_d, ac_d, dgw_d, dsk_d, qw_d,
          nid_d, w0_d, y_d, bcst_d):
    nc = tc.nc

    with (
        tc.tile_pool(name="const", bufs=1) as const,
        tc.tile_pool(name="xtcp", bufs=2) as xtcp,
        tc.tile_pool(name="xprp", bufs=1) as xprp,
        tc.tile_pool(name="pallcp", bufs=2) as pallcp,
        tc.tile_pool(name="xbp", bufs=2) as xbp,
        tc.tile_pool(name="dtp", bufs=2) as dtp,
        tc.tile_pool(name="bcp", bufs=1) as bcp,
        tc.tile_pool(name="atp", bufs=4) as atp,
        tc.tile_pool(name="gwp", bufs=2) as gwp,
        tc.tile_pool(name="dlpp", bufs=2) as dlpp,
        tc.tile_pool(name="wtpp", bufs=4) as wtpp,
        tc.tile_pool(name="sctpp", bufs=2) as sctpp,
        tc.tile_pool(name="workp", bufs=1) as workp,
        tc.tile_pool(name="qycp", bufs=2) as qycp,
        tc.tile_pool(name="youtp", bufs=1) as youtp,
        tc.tile_pool(name="psY", bufs=2, space="PSUM") as psY,
        tc.tile_pool(name="psQ", bufs=2, space="PSUM") as psQ,
        tc.tile_pool(name="psP", bufs=2, space="PSUM") as psP,
    ):
        # ---- constants ----
        nident = const.tile([128, 128], FP16)
        nc.scalar.dma_start(nident, nid_d[:, :])
        wall = const.tile([128, KD, 112], FP16)
        nc.sync.dma_start(wall, wall_d.ap().rearrange("p (k m) -> p k m",
                                                      k=KD))
        w2r = const.tile([64, NDT, 128], FP16)
        nc.scalar.dma_start(w2r, w2_d.ap().rearrange("p (d m) -> p d m",
                                                     d=NDT))
        bdt2 = const.tile([128, NDT], F32)
        nc.scalar.dma_start(bdt2, bd_d[:, :])
        acols = const.tile([128, NDT * N], F32)
        nc.scalar.dma_start(acols, ac_d[:, :])
        dskw = const.tile([128, NDT, 128], FP16)
        nc.sync.dma_start(dskw, dsk_d.ap().rearrange("p (d m) -> p d m",
                                                     d=NDT))
        qw = const.tile([16, NDT, 128], FP16)
        nc.sync.dma_start(qw, qw_d.ap().rearrange("p (d m) -> p d m",
                                                  d=NDT))
        dgw = const.tile([128, NDT * N, 128], FP16)
        nc.gpsimd.dma_start(dgw, dgw_d.ap().rearrange("p (g m) -> p g m",
                                                      g=NDT * N))
        wc = const.tile([128, NDT * N], F32)
        nc.sync.dma_start(wc, w0_d[:, :])


        stage_prev = {}
        dts_tiles = {}
        deferred = []

        def flush_deferred():
            while deferred:
                deferred.pop(0)()

        def flush_deferred_yo():
            while deferred_yo:
                deferred_yo.pop(0)()

        qsb_pending = {}
        bc_tiles = {}

        def make_bc(ch):
            # realigned B/C rows for the q-trick (partition move 64->0);
            # emitted mid-chunk so the bc TT never head-blocks DVE's stream
            # on the next chunk's staging chain.
            pallc = stage_prev[ch][1]
            btc = workp.tile([16, TC], FP16, tag="btc", name="btc")
            nc.sync.dma_start(btc, pallc[64:80, 1:1 + TC])
            ctc = workp.tile([16, TC], FP16, tag="ctc", name="ctc")
            nc.sync.dma_start(ctc, pallc[96:112, 1:1 + TC])
            bc = workp.tile([16, TC], FP16, tag="bc", name="bc")
            nc.vector.tensor_tensor(bc, btc, ctc, AL.mult)
            bc_tiles[ch] = bc

        def prep_q(ch, dtl):
            # qsb for (ch, dtl): emitted one dtile-pass ahead so the PE
            # matmuls sit mid-stream, never behind a dtile tail.
            bcq = bc_tiles[ch]
            qsb = qycp.tile([128, TC], FP16, tag="qsb", name="qsb")
            for hf in range(NPC):
                pq = psQ.tile([128, PH], F32, tag="psQ")
                nc.tensor.matmul(pq, qw[:, dtl, :],
                                 bcq[:, hf * PH:(hf + 1) * PH],
                                 start=True, stop=True)
                nc.scalar.copy(qsb[:, hf * PH:(hf + 1) * PH], pq)
            qsb_pending[(ch, dtl)] = qsb

        def dt_stage(ch, dtl):
            # dt for one dtile: softplus(w2 @ xr + b)
            if ch not in dts_tiles:
                dts_tiles[ch] = dtp.tile([128, NDT, TC], FP16, tag="dts",
                                         name="dts")
            dts = dts_tiles[ch]
            pallc = stage_prev[ch][1]
            for hf in range(NPC):
                sl = slice(1 + hf * PH, 1 + (hf + 1) * PH)
                pdt = psQ.tile([128, PH], F32, tag="psQ")
                nc.tensor.matmul(pdt, w2r[:, dtl, :], pallc[0:64, sl],
                                 start=True, stop=True)
                dsl = dts[:, dtl, hf * PH:(hf + 1) * PH]
                nc.scalar.activation(dsl, pdt, AF.Exp,
                                     bias=bdt2[:, dtl:dtl + 1], scale=1.0)
            nc.scalar.activation(dts[:, dtl, :], dts[:, dtl, :],
                                 AF.Ln, bias=1.0, scale=1.0)

        def stage(ch):
            """Load + transpose x for chunk ch, run projections, ship B/C
            rows to the DRAM staging ring."""
            t0 = ch * TC
            ring = ch % 2
            xtc = xtcp.tile([128, NDT, TC], FP16, tag="xtc", name="xtc")
            xpr = xprp.tile([128, KD - NDT, TC], FP16, tag="xpr", name="xpr")
            pallc = pallcp.tile([112, TC + 1], FP16, tag="pallc",
                                name="pallc")
            for k in range(KD):
                src = x_d[t0:t0 + TC, k * 128:(k + 1) * 128]
                if k < NDT:
                    nc.sync.dma_start_transpose(xtc[:, k, :], src)
                else:
                    nc.sync.dma_start_transpose(xpr[:, k - NDT, :], src)
            xb = xbp.tile([128, NDT, 1], FP16, tag="xb", name="xb")
            if ch == 0:
                nc.vector.memset(pallc[:, 0:1], 0.0)
                nc.vector.memset(xb, 0.0)
            else:
                xp0, pp0 = stage_prev[ch - 1][0], stage_prev[ch - 1][1]
                nc.scalar.copy(pallc[:, 0:1], pp0[:, TC:TC + 1])
                nc.scalar.copy(xb, xp0[:, :, TC - 1:TC])
            stage_prev[ch] = (xtc, pallc, None, xb)

            for tp in range(NPC):
                pp = psP.tile([112, PH], F32, tag="psP")
                for k in range(KD):
                    if k < NDT:
                        srck = xtc[:, k, tp * PH:(tp + 1) * PH]
                    else:
                        srck = xpr[:, k - NDT, tp * PH:(tp + 1) * PH]
                    nc.tensor.matmul(pp, wall[:, k, :], srck,
                                     start=(k == 0), stop=(k == KD - 1))
                nc.scalar.copy(pallc[:, 1 + tp * PH:1 + (tp + 1) * PH], pp)

            # ship B rows (with t0-1 col) and C rows to the DRAM ring
            nc.scalar.dma_start(bcst_d[ring * 32:ring * 32 + 16, :],
                                pallc[64:80, :])
            nc.scalar.dma_start(bcst_d[ring * 32 + 16:ring * 32 + 32, 0:TC],
                                pallc[96:112, 1:TC + 1])
            stage_prev[ch] = (xtc, pallc, None, xb)

        def bcast_pair(ch, pr):
            """DMA-broadcast B/C rows for pair pr of chunk ch from the DRAM
            ring to all 128 partitions."""
            ring = ch % 2
            brp = bcp.tile([128, 2, TC + 1], FP16, tag=f"brp{pr}",
                           name=f"brp{pr}")
            nc.sync.dma_start(
                brp, bcst_d[ring * 32 + 2 * pr:ring * 32 + 2 * pr + 2, :]
                .unsqueeze(0).broadcast_to([128, 2, TC + 1]))
            crp = bcp.tile([128, 2, TC], FP16, tag=f"crp{pr}",
                           name=f"crp{pr}")
            nc.sync.dma_start(
                crp, bcst_d[ring * 32 + 16 + 2 * pr:ring * 32 + 18 + 2 * pr,
                            0:TC]
                .unsqueeze(0).broadcast_to([128, 2, TC]))
            return brp, crp

        def run_chunk(ch):
            t0 = ch * TC
            xtc, pallc, _, xb = stage_prev[ch]
            dts = dts_tiles[ch]

            pair_tiles = {}
            for dtl in range(NDT):
                py = psY.tile([128, TC], F32, tag="psY", name="py")
                pys = [py[:, hf * PH:(hf + 1) * PH] for hf in range(NPC)]
                # q path: pq matmuls now; qsb copies deferred into ACT slack
                # after the next pair's at-exps; ycr emitted at pr==2 so it
                # does not head-block Pool's sct stream.
                if (ch, dtl) not in qsb_pending:
                    prep_q(ch, dtl)
                qsb = qsb_pending.pop((ch, dtl))
                ycr = qycp.tile([128, TC], FP16, tag="ycr", name="ycr")

                for pr in range(NPR):
                    if dtl == 0:
                        pair_tiles[pr] = bcast_pair(ch, pr)
                    brp, crp = pair_tiles[pr]
                    g0 = dtl * N + 2 * pr

                    ats = []
                    for j in range(2):
                        at = atp.tile([128, TC], F32, tag="at", name="at")
                        nc.scalar.activation(
                            at, dts[:, dtl, :], AF.Exp,
                            scale=acols[:, g0 + j:g0 + j + 1])
                        ats.append(at)
                    if pr == 1:
                        flush_deferred()
                        if dtl == 2 and ch + 1 < NCH:
                            make_bc(ch + 1)
                    elif pr == 3 and dtl == 0 and ch + 1 < NCH:
                        stage(ch + 1)
                    elif pr == 3:
                        flush_deferred_yo()
                    elif pr == 5 and ch + 1 < NCH:
                        dt_stage(ch + 1, dtl)
                    elif pr == 6:
                        if dtl + 1 < NDT:
                            prep_q(ch, dtl + 1)
                        elif ch + 1 < NCH:
                            prep_q(ch + 1, 0)

                    gtp = gwp.tile([128, 2, TC], FP16, tag="gtp",
                                   name="gtp")
                    nc.vector.tensor_tensor(
                        gtp,
                        xtc[:, dtl, :].unsqueeze(1)
                        .broadcast_to([128, 2, TC]),
                        brp[:, :, 1:TC + 1], AL.mult)
                    # boundary gt at time t0-1 from the previous chunk's x
                    gb = gwp.tile([128, 2, 1], FP16, tag="gb", name="gb")
                    nc.vector.tensor_tensor(
                        gb, xb[:, dtl, :].unsqueeze(1)
                        .broadcast_to([128, 2, 1]),
                        brp[:, :, 0:1], AL.mult)
                    if dtl == NDT - 1 and ch + 1 < NCH:
                        pair_tiles[(ch + 1, pr)] = bcast_pair(ch + 1, pr)
                    dlp = dlpp.tile([128, 2, TC], FP16, tag="dlp", name="dlp")
                    dl_eng = nc.gpsimd if pr in DL_POOL else nc.vector
                    nc.vector.tensor_tensor(dlp[:, :, 0:1], gtp[:, :, 0:1],
                                            gb, AL.subtract)
                    dl_eng.tensor_tensor(dlp[:, :, 1:TC], gtp[:, :, 1:TC],
                                         gtp[:, :, 0:TC - 1], AL.subtract)

                    wtp = wtpp.tile([128, 2, TC], FP16, tag="wtp",
                                    name="wtp")
                    for j in range(2):
                        nc.vector.tensor_tensor_scan(
                            wtp[:, j, :], dlp[:, j, :], ats[j],
                            wc[:, g0 + j:g0 + j + 1],
                            op0=AL.add, op1=AL.mult)
                        nc.scalar.copy(wc[:, g0 + j:g0 + j + 1],
                                       wtp[:, j, TC - 1:TC])

                    if pr == 3:
                        nc.gpsimd.tensor_tensor(
                            ycr, xtc[:, dtl, :], qsb, AL.mult)
                    sctp = sctpp.tile([128, 2, TC], FP16, tag="sctp",
                                      name="sctp")
                    last_pass = ch == NCH - 1 and dtl == NDT - 1 and pr >= 6
                    sct_eng = (nc.vector if last_pass else
                               nc.gpsimd if pr in SCT_POOL else nc.vector)
                    sct_eng.tensor_tensor(sctp, wtp, crp, AL.mult)

                    for j in range(2):
                        for hf in range(NPC):
                            nc.tensor.matmul(
                                pys[hf], dgw[:, g0 + j, :],
                                sctp[:, j, hf * PH:(hf + 1) * PH],
                                start=(pr == 0 and j == 0), stop=False)

                # D_skip * x and the -x*q correction
                for hf in range(NPC):
                    nc.tensor.matmul(
                        pys[hf], dskw[:, dtl, :],
                        xtc[:, dtl, hf * PH:(hf + 1) * PH],
                        start=False, stop=False)
                for hf in range(NPC):
                    nc.tensor.matmul(pys[hf], nident,
                                     ycr[:, hf * PH:(hf + 1) * PH],
                                     start=False, stop=True)

                def make_yo(py=py, dtl=dtl, t0=t0):
                    def emit():
                        yo = youtp.tile([128, TC], FP16, tag="yo", name="yo")
                        nc.scalar.copy(yo, py)
                        nc.sync.dma_start(
                            y_d[dtl * 128:(dtl + 1) * 128, t0:t0 + TC], yo)
                    return emit
                deferred_yo.append(make_yo())
                # chunk 0 still hoists its own dt softplus dtile-to-dtile;
                # later chunks were fully precomputed at pr==5 of chunk ch-1.
                if ch == 0 and dtl + 1 < NDT:
                    dt_stage(ch, dtl + 1)

        pe_warmup()
        stage(0)
        make_bc(0)
        dt_stage(0, 0)
        for ch in range(NCH):
            run_chunk(ch)
        flush_deferred()
        flush_deferred_yo()


def kernel(x, state, log_A, W_B, W_C, W_dt1, W_dt2, b_dt2, D_skip):
    if "nc" not in _CACHE:
        _CACHE["nc"] = _build_program()
    nc = _CACHE["nc"]

    x = np.asarray(x, np.float32)
    state = np.asarray(state, np.float32)
    A = (-np.exp(np.asarray(log_A, np.float32))).astype(np.float32)
    G = (A + np.float32(1e-8)).astype(np.float32)
    invG = (np.float32(1.0) / G).astype(np.float32)
    W_B = np.asarray(W_B, np.float32)
    W_C = np.asarray(W_C, np.float32)
    W_dt1 = np.asarray(W_dt1, np.float32)
    W_dt2 = np.asarray(W_dt2, np.float32)
    b_dt2 = np.asarray(b_dt2, np.float32)
    D_skip = np.asarray(D_skip, np.float32)

    nident = (-np.eye(128)).astype(np.float16)

    in_maps = []
    for c in range(NCORES):
        b, h = c // 2, c % 2
        loc = slice(h * DH, (h + 1) * DH)
        perm = np.r_[np.arange(h * DH, (h + 1) * DH),
                     np.arange((1 - h) * DH, (2 - h) * DH)]
        Al = A[loc]                      # [DH, N]
        Gl = G[loc]
        invGl = invG[loc]

        # wall: [W_dt1.T | W_B.T | zeros | W_C.T] with permuted rows
        wallf = np.concatenate(
            [W_dt1.T[perm], W_B.T[perm],
             np.zeros((D, 16), np.float32), W_C.T[perm]], axis=1)  # [D, 112]
        wall = np.ascontiguousarray(
            wallf.reshape(KD, 128, 112).transpose(1, 0, 2).reshape(
                128, KD * 112)).astype(np.float16)

        # w2r: [64, NDT*128]
        w2r = np.ascontiguousarray(
            W_dt2[loc].T.reshape(64, NDT, 128).reshape(64, NDT * 128)
        ).astype(np.float16)

        bd = np.ascontiguousarray(b_dt2[loc].reshape(NDT, 128).T)

        # acols: [128, NDT*N] col (dtl*N+n) = A[dtl*128+p, n]
        acols = np.ascontiguousarray(
            Al.reshape(NDT, 128, N).transpose(1, 0, 2).reshape(128, NDT * N))

        # dgw: diag(invG) per (dtl, n): [128, NDT*N*128]
        dgwm = np.zeros((128, NDT * N, 128), np.float32)
        p = np.arange(128)
        for dtl in range(NDT):
            for n in range(N):
                dgwm[p, dtl * N + n, p] = invGl[dtl * 128 + p, n]
        dgw = np.ascontiguousarray(
            dgwm.reshape(128, NDT * N * 128)).astype(np.float16)

        # dskw: diag(D_skip) per dtl
        dskm = np.zeros((128, NDT, 128), np.float32)
        for dtl in range(NDT):
            dskm[p, dtl, p] = D_skip[loc][dtl * 128 + p]
        dskw = np.ascontiguousarray(
            dskm.reshape(128, NDT * 128)).astype(np.float16)

        # qw: [16, NDT*128]  qw[n, dtl*128+p] = invG[dtl*128+p, n]
        qwm = np.ascontiguousarray(
            invGl.T.reshape(N, NDT, 128).reshape(16, NDT * 128)
        ).astype(np.float16)

        # w0init: G*state0 laid out [128, NDT*N] (fp16 carries)
        w0 = (Gl * state[b, loc]).reshape(NDT, 128, N).transpose(1, 0, 2)
        w0 = np.ascontiguousarray(w0.reshape(128, NDT * N)).astype(np.float32)

        in_maps.append({
            "x16": np.ascontiguousarray(x[b][:, perm]).astype(np.float16),
            "wall": wall,
            "w2r": w2r,
            "bdt2": bd,
            "acols": acols,
            "dgw": dgw,
            "dskw": dskw,
            "qw": qwm,
            "nident": nident,
            "w0init": w0,
        })

    res = run_bass_kernel_spmd(nc, in_maps, core_ids=list(range(NCORES)))

    y = np.empty((B, T, D), np.float32)
    for c in range(NCORES):
        b, h = c // 2, c % 2
        y[b][:, h * DH:(h + 1) * DH] = res.results[c]["yT"].T.astype(
            np.float32)
    return y


# revision 71
# speedup vs baseline: 1.0040x; 1.0003x over previous
"""Mamba-1 selective scan on 8 Trainium2 NeuronCores — v2.

Sharding: core c -> (batch b = c//2, D-half h = c%2): each core owns 512
channels of one batch for the recurrence; projections need the full D=1024.

Math (exact ZOH, rescaled state):
  G = A + 1e-8,  shat := G * s
  a_t = exp(dt_t * A)                           (per d,n,t)
  shat_t = a_t shat_{t-1} + (a_t - 1) ghat_t,   ghat = x * B
  w := shat + ghat  ->  w_t = (delta_t + w_{t-1}) * a_t,
       delta_t = ghat_t - ghat_{t-1}            (hw tensor_tensor_scan)
  y_t[d] = sum_n (1/G)[d,n] (w - ghat) C[n,t] + Dskip[d] x[d,t]
         = [sum_n diag(1/G_n) @ (w_n * crep_n)]  - x*q + Dskip*x
    q[d,t] = sum_n (1/G)[d,n] B[n,t] C[n,t]     (PE matmul of bc = B*C)

v2 engine plan (vs the v1 PE-broadcast/ACT-copy design):
  - x arrives transposed via XBAR dma_start_transpose (no PE transposes,
    no psum staging copies).
  - B/C rows bounce through a DRAM ring and come back as DMA partition
    broadcasts (no PE broadcast matmuls, no ACT psum->sbuf copies).
  - n is processed in pairs; gt/dl/sct are single [128, 2, TC] tensor ops
    (0-stride broadcast of x over the pair dim).
  - dl/sct alternate pairs between DVE and Pool to balance the two engines;
    scans are DVE-only (ISA).
  - scan carries live in a fp16 wc array updated by tiny DMAs, not ACT.
"""

import sys

import numpy as np

sys.path.insert(0, "/opt/trn_rl_repo")

import concourse.bacc as bacc
import concourse.mybir as mybir
import concourse.tile as tile
from concourse.bass_utils import run_bass_kernel_spmd

B, T, D, N, R = 4, 4096, 1024, 16, 64
NCORES = 8
DH = D // 2            # channels per core
NDT = DH // 128        # d-tiles per core (4)
KD = D // 128          # k-tiles over full D for projections (8)
TC = 1024              # time chunk
NCH = T // TC
PH = 512               # psum piece (one bank of f32)
NPC = TC // PH         # psum pieces per chunk (2)
NPR = N // 2           # n-pairs (8)
F32 = mybir.dt.float32
FP16 = mybir.dt.float16
AL = mybir.AluOpType
AF = mybir.ActivationFunctionType

# pair index sets: which pairs run dl / sct on Pool (else DVE).
# dl stays on DVE (it feeds the scans: keep the DVE chain self-contained);
# sct is a leaf (only the PE reads it) so it all goes to Pool.
DL_POOL = ()
SCT_POOL = (0, 1, 2, 3, 4, 5, 6, 7)

_CACHE = {}


def _patch_act_tables():
    """Route Exp+Ln to natural_log_exp_and_others so the softplus (Exp,Ln)
    and the main-loop Exp never force activation-table reloads."""
    import concourse.bacc as _bacc
    from concourse.hw_specs import get_activation_tables as _orig

    def patched(arch):
        t = _orig(arch)
        exp = mybir.ActivationFunctionType.Exp
        ln = mybir.ActivationFunctionType.Ln
        for name, fns in t.items():
            if name != "natural_log_exp_and_others":
                fns.discard(exp)
                fns.discard(ln)
        return t

    _bacc.get_activation_tables = patched


def _build_program():
    _patch_act_tables()
    nc = bacc.Bacc(
        "TRN2",
        target_bir_lowering=False,
        debug=False,
        num_devices=NCORES,
    )

    x_d = nc.dram_tensor("x16", [T, D], FP16, kind="ExternalInput")
    wall_d = nc.dram_tensor("wall", [128, KD * 112], FP16, kind="ExternalInput")
    w2_d = nc.dram_tensor("w2r", [64, NDT * 128], FP16, kind="ExternalInput")
    bd_d = nc.dram_tensor("bdt2", [128, NDT], F32, kind="ExternalInput")
    ac_d = nc.dram_tensor("acols", [128, NDT * N], F32, kind="ExternalInput")
    dgw_d = nc.dram_tensor("dgw", [128, NDT * N * 128], FP16,
                           kind="ExternalInput")
    dsk_d = nc.dram_tensor("dskw", [128, NDT * 128], FP16,
                           kind="ExternalInput")
    qw_d = nc.dram_tensor("qw", [16, NDT * 128], FP16, kind="ExternalInput")
    nid_d = nc.dram_tensor("nident", [128, 128], FP16, kind="ExternalInput")
    w0_d = nc.dram_tensor("w0init", [128, NDT * N], F32, kind="ExternalInput")
    y_d = nc.dram_tensor("yT", [DH, T], FP16, kind="ExternalOutput")
    # B/C row staging ring in DRAM: per chunk 32 rows
    # rows 0..15: B rows over times t0-1 .. t0+TC-1  ([16, TC+1])
    # rows 16..31: C rows over times t0 .. t0+TC-1   ([16, TC], col TC unused)
    bcst_d = nc.dram_tensor("bcstage", [2 * 32, TC + 1], FP16, kind="Internal")

    with tile.TileContext(nc) as tc:
        _body(tc, x_d, wall_d, w2_d, bd_d, ac_d, dgw_d, dsk_d, qw_d,
              nid_d, w0_d, y_d, bcst_d)

    nc.compile()
    return nc


def _body(tc, x_d, wall_d, w2_d, bd_d, ac_d, dgw_d, dsk_d, qw_d,
          nid_d, w0_d, y_d, bcst_d):
    nc = tc.nc

    with (
        tc.tile_pool(name="const", bufs=1) as const,
        tc.tile_pool(name="xtcp", bufs=2) as xtcp,
        tc.tile_pool(name="xprp", bufs=1) as xprp,
        tc.tile_pool(name="pallcp", bufs=2) as pallcp,
        tc.tile_pool(name="xbp", bufs=2) as xbp,
        tc.tile_pool(name="dtp", bufs=2) as dtp,
        tc.tile_pool(name="bcp", bufs=1) as bcp,
        tc.tile_pool(name="atp", bufs=4) as atp,
        tc.tile_pool(name="gwp", bufs=2) as gwp,
        tc.tile_pool(name="dlpp", bufs=2) as dlpp,
        tc.tile_pool(name="wtpp", bufs=4) as wtpp,
        tc.tile_pool(name="sctpp", bufs=2) as sctpp,
        tc.tile_pool(name="workp", bufs=1) as workp,
        tc.tile_pool(name="qycp", bufs=2) as qycp,
        tc.tile_pool(name="youtp", bufs=1) as youtp,
        tc.tile_pool(name="psY", bufs=2, space="PSUM") as psY,
        tc.tile_pool(name="psQ", bufs=3, space="PSUM") as psQ,
        tc.tile_pool(name="psP", bufs=1, space="PSUM") as psP,
    ):
        # ---- constants ----
        nident = const.tile([128, 128], FP16)
        nc.scalar.dma_start(nident, nid_d[:, :])
        wall = const.tile([128, KD, 112], FP16)
        nc.sync.dma_start(wall, wall_d.ap().rearrange("p (k m) -> p k m",
                                                      k=KD))
        w2r = const.tile([64, NDT, 128], FP16)
        nc.scalar.dma_start(w2r, w2_d.ap().rearrange("p (d m) -> p d m",
                                                     d=NDT))
        bdt2 = const.tile([128, NDT], F32)
        nc.scalar.dma_start(bdt2, bd_d[:, :])
        acols = const.tile([128, NDT * N], F32)
        nc.scalar.dma_start(acols, ac_d[:, :])
        dskw = const.tile([128, NDT, 128], FP16)
        nc.sync.dma_start(dskw, dsk_d.ap().rearrange("p (d m) -> p d m",
                                                     d=NDT))
        qw = const.tile([16, NDT, 128], FP16)
        nc.sync.dma_start(qw, qw_d.ap().rearrange("p (d m) -> p d m",
                                                  d=NDT))
        dgw = const.tile([128, NDT * N, 128], FP16)
        nc.gpsimd.dma_start(dgw, dgw_d.ap().rearrange("p (g m) -> p g m",
                                                      g=NDT * N))
        wc = const.tile([128, NDT * N], F32)
        nc.sync.dma_start(wc, w0_d[:, :])


        stage_prev = {}
        dts_tiles = {}
        deferred = []

        def flush_deferred():
            while deferred:
                deferred.pop(0)()

        def flush_deferred_yo():
            while deferred_yo:
                deferred_yo.pop(0)()

        qsb_pending = {}
        bc_tiles = {}

        def make_bc(ch):
            # realigned B/C rows for the q-trick (partition move 64->0);
            # emitted mid-chunk so the bc TT never head-blocks DVE's stream
            # on the next chunk's staging chain.
            pallc = stage_prev[ch][1]
            btc = workp.tile([16, TC], FP16, tag="btc", name="btc")
            nc.sync.dma_start(btc, pallc[64:80, 1:1 + TC])
            ctc = workp.tile([16, TC], FP16, tag="ctc", name="ctc")
            nc.sync.dma_start(ctc, pallc[96:112, 1:1 + TC])
            bc = workp.tile([16, TC], FP16, tag="bc", name="bc")
            nc.vector.tensor_tensor(bc, btc, ctc, AL.mult)
            bc_tiles[ch] = bc

        def prep_q(ch, dtl):
            # qsb for (ch, dtl): emitted one dtile-pass ahead so the PE
            # matmuls sit mid-stream, never behind a dtile tail.
            bcq = bc_tiles[ch]
            qsb = qycp.tile([128, TC], FP16, tag="qsb", name="qsb")
            for hf in range(NPC):
                pq = psQ.tile([128, PH], F32, tag="psQ")
                nc.tensor.matmul(pq, qw[:, dtl, :],
                                 bcq[:, hf * PH:(hf + 1) * PH],
                                 start=True, stop=True)
                nc.scalar.copy(qsb[:, hf * PH:(hf + 1) * PH], pq)
            qsb_pending[(ch, dtl)] = qsb

        def dt_stage(ch, dtl):
            # dt for one dtile: softplus(w2 @ xr + b)
            if ch not in dts_tiles:
                dts_tiles[ch] = dtp.tile([128, NDT, TC], FP16, tag="dts",
                                         name="dts")
            dts = dts_tiles[ch]
            pallc = stage_prev[ch][1]
            for hf in range(NPC):
                sl = slice(1 + hf * PH, 1 + (hf + 1) * PH)
                pdt = psQ.tile([128, PH], F32, tag="psQ")
                nc.tensor.matmul(pdt, w2r[:, dtl, :], pallc[0:64, sl],
                                 start=True, stop=True)
                dsl = dts[:, dtl, hf * PH:(hf + 1) * PH]
                nc.scalar.activation(dsl, pdt, AF.Exp,
                                     bias=bdt2[:, dtl:dtl + 1], scale=1.0)
            nc.scalar.activation(dts[:, dtl, :], dts[:, dtl, :],
                                 AF.Ln, bias=1.0, scale=1.0)

        def stage(ch):
            """Load + transpose x for chunk ch, run projections, ship B/C
            rows to the DRAM staging ring."""
            t0 = ch * TC
            ring = ch % 2
            xtc = xtcp.tile([128, NDT, TC], FP16, tag="xtc", name="xtc")
            xpr = xprp.tile([128, KD - NDT, TC], FP16, tag="xpr", name="xpr")
            pallc = pallcp.tile([112, TC + 1], FP16, tag="pallc",
                                name="pallc")
            for k in range(KD):
                src = x_d[t0:t0 + TC, k * 128:(k + 1) * 128]
                if k < NDT:
                    nc.sync.dma_start_transpose(xtc[:, k, :], src)
                else:
                    nc.sync.dma_start_transpose(xpr[:, k - NDT, :], src)
            xb = xbp.tile([128, NDT, 1], FP16, tag="xb", name="xb")
            if ch == 0:
                nc.vector.memset(pallc[:, 0:1], 0.0)
                nc.vector.memset(xb, 0.0)
            else:
                xp0, pp0 = stage_prev[ch - 1][0], stage_prev[ch - 1][1]
                nc.scalar.copy(pallc[:, 0:1], pp0[:, TC:TC + 1])
                nc.scalar.copy(xb, xp0[:, :, TC - 1:TC])
            stage_prev[ch] = (xtc, pallc, None, xb)

            for tp in range(NPC):
                pp = psP.tile([112, PH], F32, tag="psP")
                for k in range(KD):
                    if k < NDT:
                        srck = xtc[:, k, tp * PH:(tp + 1) * PH]
                    else:
                        srck = xpr[:, k - NDT, tp * PH:(tp + 1) * PH]
                    nc.tensor.matmul(pp, wall[:, k, :], srck,
                                     start=(k == 0), stop=(k == KD - 1))
                nc.scalar.copy(pallc[:, 1 + tp * PH:1 + (tp + 1) * PH], pp)

            # ship B rows (with t0-1 col) and C rows to the DRAM ring
            nc.scalar.dma_start(bcst_d[ring * 32:ring * 32 + 16, :],
                                pallc[64:80, :])
            nc.scalar.dma_start(bcst_d[ring * 32 + 16:ring * 32 + 32, 0:TC],
                                pallc[96:112, 1:TC + 1])
            stage_prev[ch] = (xtc, pallc, None, xb)

        def bcast_pair(ch, pr):
            """DMA-broadcast B/C rows for pair pr of chunk ch from the DRAM
            ring to all 128 partitions."""
            ring = ch % 2
            brp = bcp.tile([128, 2, TC + 1], FP16, tag=f"brp{pr}",
                           name=f"brp{pr}")
            nc.sync.dma_start(
                brp, bcst_d[ring * 32 + 2 * pr:ring * 32 + 2 * pr + 2, :]
                .unsqueeze(0).broadcast_to([128, 2, TC + 1]))
            crp = bcp.tile([128, 2, TC], FP16, tag=f"crp{pr}",
                           name=f"crp{pr}")
            nc.sync.dma_start(
                crp, bcst_d[ring * 32 + 16 + 2 * pr:ring * 32 + 18 + 2 * pr,
                            0:TC]
                .unsqueeze(0).broadcast_to([128, 2, TC]))
            return brp, crp

        def run_chunk(ch):
            t0 = ch * TC
            xtc, pallc, _, xb = stage_prev[ch]
            dts = dts_tiles[ch]

            pair_tiles = {}
            for dtl in range(NDT):
                py = psY.tile([128, TC], F32, tag="psY", name="py")
                pys = [py[:, hf * PH:(hf + 1) * PH] for hf in range(NPC)]
                # q path: pq matmuls now; qsb copies deferred into ACT slack
                # after the next pair's at-exps; ycr emitted at pr==2 so it
                # does not head-block Pool's sct stream.
                if (ch, dtl) not in qsb_pending:
                    prep_q(ch, dtl)
                qsb = qsb_pending.pop((ch, dtl))
                ycr = qycp.tile([128, TC], FP16, tag="ycr", name="ycr")

                for pr in range(NPR):
                    if dtl == 0:
                        pair_tiles[pr] = bcast_pair(ch, pr)
                    brp, crp = pair_tiles[pr]
                    g0 = dtl * N + 2 * pr

                    ats = []
                    for j in range(2):
                        at = atp.tile([128, TC], F32, tag="at", name="at")
                        nc.scalar.activation(
                            at, dts[:, dtl, :], AF.Exp,
                            scale=acols[:, g0 + j:g0 + j + 1])
                        ats.append(at)
                    if pr == 1:
                        flush_deferred()
                        if dtl == 2 and ch + 1 < NCH:
                            make_bc(ch + 1)
                    elif pr == 3 and dtl == 0 and ch + 1 < NCH:
                        stage(ch + 1)
                    elif pr == 3:
                        flush_deferred_yo()
                    elif pr == 5 and ch + 1 < NCH:
                        dt_stage(ch + 1, dtl)
                    elif pr == 6:
                        if dtl + 1 < NDT:
                            prep_q(ch, dtl + 1)
                        elif ch + 1 < NCH:
                            prep_q(ch + 1, 0)

                    gtp = gwp.tile([128, 2, TC], FP16, tag="gtp",
                                   name="gtp")
                    nc.vector.tensor_tensor(
                        gtp,
                        xtc[:, dtl, :].unsqueeze(1)
                        .broadcast_to([128, 2, TC]),
                        brp[:, :, 1:TC + 1], AL.mult)
                    # boundary gt at time t0-1 from the previous chunk's x
                    gb = gwp.tile([128, 2, 1], FP16, tag="gb", name="gb")
                    nc.vector.tensor_tensor(
                        gb, xb[:, dtl, :].unsqueeze(1)
                        .broadcast_to([128, 2, 1]),
                        brp[:, :, 0:1], AL.mult)
                    if dtl == NDT - 1 and ch + 1 < NCH:
                        pair_tiles[(ch + 1, pr)] = bcast_pair(ch + 1, pr)
                    dlp = dlpp.tile([128, 2, TC], FP16, tag="dlp", name="dlp")
                    dl_eng = nc.gpsimd if pr in DL_POOL else nc.vector
                    nc.vector.tensor_tensor(dlp[:, :, 0:1], gtp[:, :, 0:1],
                                            gb, AL.subtract)
                    dl_eng.tensor_tensor(dlp[:, :, 1:TC], gtp[:, :, 1:TC],
                                         gtp[:, :, 0:TC - 1], AL.subtract)

                    wtp = wtpp.tile([128, 2, TC], FP16, tag="wtp",
                                    name="wtp")
                    for j in range(2):
                        nc.vector.tensor_tensor_scan(
                            wtp[:, j, :], dlp[:, j, :], ats[j],
                            wc[:, g0 + j:g0 + j + 1],
                            op0=AL.add, op1=AL.mult)
                        nc.scalar.copy(wc[:, g0 + j:g0 + j + 1],
                                       wtp[:, j, TC - 1:TC])

                    if pr == 3:
                        nc.gpsimd.tensor_tensor(
                            ycr, xtc[:, dtl, :], qsb, AL.mult)
                    sctp = sctpp.tile([128, 2, TC], FP16, tag="sctp",
                                      name="sctp")
                    last_pass = ch == NCH - 1 and dtl == NDT - 1 and pr >= 6
                    sct_eng = (nc.vector if last_pass else
                               nc.gpsimd if pr in SCT_POOL else nc.vector)
                    sct_eng.tensor_tensor(sctp, wtp, crp, AL.mult)

                    for j in range(2):
                        for hf in range(NPC):
                            nc.tensor.matmul(
                                pys[hf], dgw[:, g0 + j, :],
                                sctp[:, j, hf * PH:(hf + 1) * PH],
                                start=(pr == 0 and j == 0), stop=False)

                # D_skip * x and the -x*q correction
                for hf in range(NPC):
                    nc.tensor.matmul(
                        pys[hf], dskw[:, dtl, :],
                        xtc[:, dtl, hf * PH:(hf + 1) * PH],
                        start=False, stop=False)
                for hf in range(NPC):
                    nc.tensor.matmul(pys[hf], nident,
                                     ycr[:, hf * PH:(hf + 1) * PH],
                                     start=False, stop=True)

                def make_yo(py=py, dtl=dtl, t0=t0):
                    def emit():
                        yo = youtp.tile([128, TC], FP16, tag="yo", name="yo")
                        nc.scalar.copy(yo, py)
                        nc.sync.dma_start(
                            y_d[dtl * 128:(dtl + 1) * 128, t0:t0 + TC], yo)
                    return emit
                deferred_yo.append(make_yo())
                # chunk 0 still hoists its own dt softplus dtile-to-dtile;
                # later chunks were fully precomputed at pr==5 of chunk ch-1.
                if ch == 0 and dtl + 1 < NDT:
                    dt_stage(ch, dtl + 1)

        pe_warmup()
        stage(0)
        make_bc(0)
        dt_stage(0, 0)
        for ch in range(NCH):
            run_chunk(ch)
        flush_deferred()
        flush_deferred_yo()


def kernel(x, state, log_A, W_B, W_C, W_dt1, W_dt2, b_dt2, D_skip):
    if "nc" not in _CACHE:
        _CACHE["nc"] = _build_program()
    nc = _CACHE["nc"]

    x = np.asarray(x, np.float32)
    state = np.asarray(state, np.float32)
    A = (-np.exp(np.asarray(log_A, np.float32))).astype(np.float32)
    G = (A + np.float32(1e-8)).astype(np.float32)
    invG = (np.float32(1.0) / G).astype(np.float32)
    W_B = np.asarray(W_B, np.float32)
    W_C = np.asarray(W_C, np.float32)
    W_dt1 = np.asarray(W_dt1, np.float32)
    W_dt2 = np.asarray(W_dt2, np.float32)
    b_dt2 = np.asarray(b_dt2, np.float32)
    D_skip = np.asarray(D_skip, np.float32)

    nident = (-np.eye(128)).astype(np.float16)

    in_maps = []
    for c in range(NCORES):
        b, h = c // 2, c % 2
        loc = slice(h * DH, (h + 1) * DH)
        perm = np.r_[np.arange(h * DH, (h + 1) * DH),
                     np.arange((1 - h) * DH, (2 - h) * DH)]
        Al = A[loc]                      # [DH, N]
        Gl = G[loc]
        invGl = invG[loc]

        # wall: [W_dt1.T | W_B.T | zeros | W_C.T] with permuted rows
        wallf = np.concatenate(
            [W_dt1.T[perm], W_B.T[perm],
             np.zeros((D, 16), np.float32), W_C.T[perm]], axis=1)  # [D, 112]
        wall = np.ascontiguousarray(
            wallf.reshape(KD, 128, 112).transpose(1, 0, 2).reshape(
                128, KD * 112)).astype(np.float16)

        # w2r: [64, NDT*128]
        w2r = np.ascontiguousarray(
            W_dt2[loc].T.reshape(64, NDT, 128).reshape(64, NDT * 128)
        ).astype(np.float16)

        bd = np.ascontiguousarray(b_dt2[loc].reshape(NDT, 128).T)

        # acols: [128, NDT*N] col (dtl*N+n) = A[dtl*128+p, n]
        acols = np.ascontiguousarray(
            Al.reshape(NDT, 128, N).transpose(1, 0, 2).reshape(128, NDT * N))

        # dgw: diag(invG) per (dtl, n): [128, NDT*N*128]
        dgwm = np.zeros((128, NDT * N, 128), np.float32)
        p = np.arange(128)
        for dtl in range(NDT):
            for n in range(N):
                dgwm[p, dtl * N + n, p] = invGl[dtl * 128 + p, n]
        dgw = np.ascontiguousarray(
            dgwm.reshape(128, NDT * N * 128)).astype(np.float16)

        # dskw: diag(D_skip) per dtl
        dskm = np.zeros((128, NDT, 128), np.float32)
        for dtl in range(NDT):
            dskm[p, dtl, p] = D_skip[loc][dtl * 128 + p]
        dskw = np.ascontiguousarray(
            dskm.reshape(128, NDT * 128)).astype(np.float16)

        # qw: [16, NDT*128]  qw[n, dtl*128+p] = invG[dtl*128+p, n]
        qwm = np.ascontiguousarray(
            invGl.T.reshape(N, NDT, 128).reshape(16, NDT * 128)
        ).astype(np.float16)

        # w0init: G*state0 laid out [128, NDT*N] (fp16 carries)
        w0 = (Gl * state[b, loc]).reshape(NDT, 128, N).transpose(1, 0, 2)
        w0 = np.ascontiguousarray(w0.reshape(128, NDT * N)).astype(np.float32)

        in_maps.append({
            "x16": np.ascontiguousarray(x[b][:, perm]).astype(np.float16),
            "wall": wall,
            "w2r": w2r,
            "bdt2": bd,
            "acols": acols,
            "dgw": dgw,
            "dskw": dskw,
            "qw": qwm,
            "nident": nident,
            "w0init": w0,
        })

    res = run_bass_kernel_spmd(nc, in_maps, core_ids=list(range(NCORES)))

    y = np.empty((B, T, D), np.float32)
    for c in range(NCORES):
        b, h = c // 2, c % 2
        y[b][:, h * DH:(h + 1) * DH] = res.results[c]["yT"].T.astype(
            np.float32)
    return y
